# revision 1
# baseline (speedup 1.0000x reference)
"""TRN2 Bass kernel for nn_BatchDenseGAT (2-layer dense GAT, bs=32, n=512).

Sharding: data-parallel over the 32 graphs -> 4 graphs per NeuronCore x 8
cores, params replicated. Host does embedding gather/concat/transpose and
mask packing; all model math runs on device.

Device strategy per graph (see inline comments):
  h_prime (n-major) and h_primeT (o-major) via fp32 matmuls (fp32r gave
  schedule-dependent corruption on HW; fp32 is wall-clock-free here since
  the scalar engine is the bottleneck); tanh on ACT;
  s/d attention projections via small matmuls; per head the [n,n] attention
  map is built in transposed layout [j,i] with ACT Prelu(s_bcast + d_col)
  then ACT Exp, masked by a bf16 adjacency multiply on DVE; out1T = hp.T@em
  accumulated in PSUM with denominators via a ones-column folded into the
  matmul lhsT (row 64 of each out1T tile); elu via
  exp(min(v,0)) + relu(v) with the "-1" folded into layer-2 weights; the
  resulting h2cT is directly f-major for layer 2, which repeats the same
  attention scheme with one head and ends in a free-dim log_softmax.
"""
import os
import sys
import numpy as np

sys.path.insert(0, '/opt/trn_rl_repo')

import ml_dtypes  # noqa: E402
import concourse.bacc as bacc  # noqa: E402
import concourse.bass as bass  # noqa: E402
import concourse.tile as tile  # noqa: E402
from concourse.tile import add_dep_helper  # noqa: E402
from concourse import mybir  # noqa: E402

F32 = mybir.dt.float32
F32R = mybir.dt.float32r
BF16 = mybir.dt.bfloat16
AF = mybir.ActivationFunctionType
BFNP = ml_dtypes.bfloat16

BS, N, NH, FO = 32, 512, 8, 64
FIN1 = 160
NCORES = 8
GPC = BS // NCORES  # graphs per core
NCH = 4             # 512 / 128 partition chunks


def build_nc():
    dbg = os.environ.get("GAT_DEBUG_DUMP", "0") == "1"
    B = 1 if os.environ.get("GAT_SERIAL", "0") == "1" else None
    dbg_g = int(os.environ.get("GAT_DEBUG_G", "0"))
    # heads whose attention pass1 runs on VectorE instead of ScalarE
    dve_h = int(os.environ.get("GAT_DVE_HEADS", "0"))
    nc = bacc.Bacc("TRN2", target_bir_lowering=False, debug=False)

    hT4 = nc.dram_tensor("hT4", [GPC, FIN1, N], F32, kind="ExternalInput")
    adjTp = nc.dram_tensor("adjTp", [GPC, 128, NCH * N], BF16,
                           kind="ExternalInput")
    w1f_d = nc.dram_tensor("w1f", [FIN1, 512], F32, kind="ExternalInput")
    asrc_d = nc.dram_tensor("asrc", [512, NH], BF16, kind="ExternalInput")
    asrep_d = nc.dram_tensor("asrep", [NCH, NH, 128, 128], BF16,
                             kind="ExternalInput")
    adst_d = nc.dram_tensor("adst", [512, NH], BF16, kind="ExternalInput")
    w2f_d = nc.dram_tensor("w2f", [512, 16], BF16, kind="ExternalInput")
    negcs_d = nc.dram_tensor("negcs", [1, 16], BF16, kind="ExternalInput")
    a2s_d = nc.dram_tensor("a2s", [16, 1], BF16, kind="ExternalInput")
    a2d_d = nc.dram_tensor("a2d", [16, 1], BF16, kind="ExternalInput")
    out_d = nc.dram_tensor("out", [GPC, N, 16], F32, kind="ExternalOutput")
    if dbg:
        dbg_t = nc.dram_tensor("dbg_t", [128, NCH * 512], BF16, kind="ExternalOutput")
        dbg_s = nc.dram_tensor("dbg_s", [NH, 512], BF16, kind="ExternalOutput")
        dbg_d = nc.dram_tensor("dbg_d", [128, NCH, NH], F32, kind="ExternalOutput")
        dbg_hpx = nc.dram_tensor("dbg_hpx", [128, NCH, NH, 65], BF16, kind="ExternalOutput")
        dbg_z = nc.dram_tensor("dbg_z", [128, NCH * 512], BF16, kind="ExternalOutput")
        dbg_em = nc.dram_tensor("dbg_em", [128, NCH * 512], BF16, kind="ExternalOutput")
        dbg_o65 = nc.dram_tensor("dbg_o65", [128, 512], F32, kind="ExternalOutput")
        dbg_rden = nc.dram_tensor("dbg_rden", [NH, 512], F32, kind="ExternalOutput")
        dbg_h2c = nc.dram_tensor("dbg_h2c", [128, NCH, 512], BF16, kind="ExternalOutput")
        dbg_lg = nc.dram_tensor("dbg_lg", [128, NCH, 16], F32, kind="ExternalOutput")

    with tile.TileContext(nc) as tc:
        with tc.tile_pool(name="consts", bufs=1) as consts, \
             tc.tile_pool(name="gbuf", bufs=(B or 3)) as gbuf, \
             tc.tile_pool(name="attn", bufs=(B or 3)) as attn, \
             tc.tile_pool(name="small", bufs=(B or 2)) as small, \
             tc.tile_pool(name="sbc", bufs=(B or 3)) as sbcp, \
             tc.tile_pool(name="elu", bufs=(B or 3)) as elup, \
             tc.tile_pool(name="fin", bufs=1) as finp, \
             tc.tile_pool(name="dram", bufs=(B or 2), space="DRAM") as dramp, \
             tc.tile_pool(name="ps_big", bufs=(B or 2), space="PSUM") as ps_big, \
             tc.tile_pool(name="ps_row", bufs=(B or 2), space="PSUM") as ps_row, \
             tc.tile_pool(name="ps_sm", bufs=(B or 2), space="PSUM") as ps_sm, \
             tc.tile_pool(name="ps_sbc", bufs=(B or 2), space="PSUM") as ps_sbc:

            # ---------- constants ----------
            w1f_a = consts.tile([128, 512], F32)
            w1f_b = consts.tile([32, 512], F32)
            nc.sync.dma_start(out=w1f_a, in_=w1f_d.ap()[0:128, :])
            nc.sync.dma_start(out=w1f_b, in_=w1f_d.ap()[128:160, :])
            asrep_sb = consts.tile([128, NCH, NH, 128], BF16)
            nc.gpsimd.dma_start(out=asrep_sb,
                                in_=asrep_d.ap().rearrange("c a p q -> p c a q"))
            asrc_sb = consts.tile([128, NCH, NH], BF16)
            adst_sb = consts.tile([128, NCH, NH], BF16)
            nc.sync.dma_start(out=asrc_sb,
                              in_=asrc_d.ap().rearrange("(c p) a -> p c a", c=NCH))
            nc.sync.dma_start(out=adst_sb,
                              in_=adst_d.ap().rearrange("(c p) a -> p c a", c=NCH))
            w2f_sb = consts.tile([128, NCH, 16], BF16)
            nc.sync.dma_start(out=w2f_sb,
                              in_=w2f_d.ap().rearrange("(c p) a -> p c a", c=NCH))
            negcs_sb = consts.tile([1, 16], BF16)
            nc.sync.dma_start(out=negcs_sb, in_=negcs_d.ap())
            a2s_sb = consts.tile([16, 1], BF16)
            a2d_sb = consts.tile([16, 1], BF16)
            nc.sync.dma_start(out=a2s_sb, in_=a2s_d.ap())
            nc.sync.dma_start(out=a2d_sb, in_=a2d_d.ap())
            onesrow = consts.tile([1, 512], BF16)
            nc.vector.memset(onesrow, 1.0)
            onescol = consts.tile([128, 1], BF16)
            nc.vector.memset(onescol, 1.0)

            logits_all = finp.tile([128, GPC, NCH, 16], F32)

            for g in range(GPC):
                # ---------- graph loads ----------
                hT_a = gbuf.tile([128, N], F32, tag="hT_a")
                hT_b = gbuf.tile([32, N], F32, tag="hT_b")
                nc.sync.dma_start(out=hT_a, in_=hT4.ap()[g, 0:128, :])
                nc.sync.dma_start(out=hT_b, in_=hT4.ap()[g, 128:160, :])
                adjT_sb = gbuf.tile([128, NCH * N], BF16, tag="adjT")
                nc.sync.dma_start(out=adjT_sb, in_=adjTp.ap()[g])

                # -- h_prime (n-major) -> hpx bf16 [j, (jc), (h, 64+ones)] ----
                hpx = gbuf.tile([128, NCH, NH, 65], BF16, tag="hpx")
                nc.vector.memset(hpx[:, :, :, 64:65], 1.0)
                for ic in range(NCH):
                    hp_ps = ps_sbc.tile([128, 512], F32, tag="sbcps")
                    nc.tensor.matmul(hp_ps[:], hT_a[:, ic * 128:(ic + 1) * 128],
                                     w1f_a[:], start=True, stop=False)
                    nc.tensor.matmul(hp_ps[:], hT_b[:, ic * 128:(ic + 1) * 128],
                                     w1f_b[:], start=False, stop=True)
                    nc.vector.tensor_copy(
                        hpx[:, ic, :, 0:64],
                        hp_ps[:].rearrange("p (h o) -> p h o", h=NH))

                # ---------- h_primeT (o-major) -> tanh -> tT bf16 ----------
                tT = gbuf.tile([128, NCH, 512], BF16, tag="tT")
                for oc in range(NCH):
                    hpT_ps = ps_sbc.tile([128, 512], F32, tag="sbcps")
                    nc.tensor.matmul(hpT_ps[:],
                                     w1f_a[:, oc * 128:(oc + 1) * 128],
                                     hT_a[:], start=True, stop=False)
                    nc.tensor.matmul(hpT_ps[:],
                                     w1f_b[:, oc * 128:(oc + 1) * 128],
                                     hT_b[:], start=False, stop=True)
                    nc.scalar.activation(tT[:, oc, :], hpT_ps[:], AF.Tanh)

                if dbg and g == dbg_g:
                    nc.sync.dma_start(out=dbg_t.ap(),
                                      in_=tT[:].rearrange("p a b -> p (a b)"))
                    nc.sync.dma_start(out=dbg_hpx.ap(), in_=hpx[:])

                # ---------- d cols [128, jc, 8] ----------
                d_sb = small.tile([128, NCH, NH], F32, tag="d_sb")
                for jc in range(NCH):
                    d_ps = ps_sbc.tile([128, 512], F32, tag="sbcps")
                    for oc in range(NCH):
                        nc.tensor.matmul(d_ps[:, 0:NH],
                                         tT[:, oc, jc * 128:(jc + 1) * 128],
                                         adst_sb[:, oc, :],
                                         start=(oc == 0), stop=(oc == NCH - 1))
                    nc.vector.tensor_copy(d_sb[:, jc, :], d_ps[:, 0:NH])

                if dbg and g == dbg_g:
                    nc.sync.dma_start(out=dbg_d.ap(), in_=d_sb[:])

                # ---------- per-head attention ----------
                # out1T per head is [65, 512]: rows 0-63 = hp_h.T @ em,
                # row 64 = denominator (ones column folded into lhsT).
                h2cT = gbuf.tile([128, NCH, 512], BF16, tag="h2cT")
                for h in range(NH):
                    # s broadcast directly from PE: each lhsT column is the
                    # same a_src chunk, so the matmul emits s replicated
                    # across all 128 partitions.
                    s_bc = ps_sbc.tile([128, 512], F32, tag="sbcps")
                    for oc in range(NCH):
                        nc.tensor.matmul(s_bc[:], asrep_sb[:, oc, h, :],
                                         tT[:, oc, :],
                                         start=(oc == 0), stop=(oc == NCH - 1))
                    z_all = attn.tile([128, NCH * 512], BF16, tag="z")
                    for jc in range(NCH):
                        zs = z_all[:, jc * 512:(jc + 1) * 512]
                        if h >= NH - dve_h:
                            # leaky on DVE: max(y, 0.2y)
                            y = elup.tile([128, 512], BF16, tag="y")
                            nc.vector.tensor_scalar_add(
                                y[:], s_bc[:], d_sb[:, jc, h:h + 1])
                            y2 = elup.tile([128, 512], BF16, tag="y2")
                            nc.vector.tensor_scalar_mul(y2[:], y[:], 0.2)
                            nc.vector.tensor_max(zs, y[:], y2[:])
                        else:
                            nc.scalar.activation(
                                zs, s_bc[:], AF.Prelu,
                                bias=d_sb[:, jc, h:h + 1],
                                scale=1.0, alpha=0.2)
                    e_all = attn.tile([128, NCH * 512], BF16, tag="e")
                    nc.scalar.activation(e_all[:], z_all[:], AF.Exp)
                    em_all = attn.tile([128, NCH * 512], BF16, tag="em")
                    nc.vector.tensor_mul(em_all[:], e_all[:], adjT_sb[:])

                    if dbg and g == dbg_g and h == 0:
                        nc.sync.dma_start(out=dbg_z.ap(), in_=z_all[:])
                        nc.sync.dma_start(out=dbg_em.ap(), in_=em_all[:])
                    o65_ps = ps_big.tile([128, 512], F32, tag="bigps")
                    for jc in range(NCH):
                        nc.tensor.matmul(
                            o65_ps[0:65, :], hpx[:, jc, h, :],
                            em_all[:, jc * 512:(jc + 1) * 512],
                            start=(jc == 0), stop=(jc == NCH - 1))

                    # -- elu: exp(min(v,0)) + relu(v); "-1" folded into w2 --
                    # min/max parts are computed per head into partition
                    # halves of pair-wide tiles; exp and the final add run
                    # once per pair on [128, 512].
                    den_sb = elup.tile([1, 512], F32, tag="densb")
                    nc.vector.tensor_copy(den_sb[:], o65_ps[64:65, :])
                    rden = elup.tile([1, 512], F32, tag="rden")
                    nc.vector.reciprocal_approx_fast(rden[:], den_sb[:])
                    if dbg and g == dbg_g:
                        nc.sync.dma_start(out=dbg_rden.ap()[h:h + 1, :],
                                          in_=rden[:])
                        if h == 0:
                            o65_sb = elup.tile([128, 512], F32, tag="dbg65")
                            nc.vector.tensor_copy(o65_sb[0:65, :],
                                                  o65_ps[0:65, :])
                            nc.sync.dma_start(out=dbg_o65.ap()[0:65, :],
                                              in_=o65_sb[0:65, :])
                    rb = elup.tile([64, 512], F32, tag="rb")
                    nc.gpsimd.partition_broadcast(rb[:], rden[:], channels=64)
                    v_t = elup.tile([64, 512], BF16, tag="v")
                    nc.vector.tensor_mul(v_t[:], o65_ps[0:64, :], rb[:])
                    prow = (h % 2) * 64
                    if h % 2 == 0:
                        m_pair = elup.tile([128, 512], BF16, tag="m")
                        p_pair = elup.tile([128, 512], BF16, tag="p")
                    nc.vector.tensor_scalar_min(m_pair[prow:prow + 64, :],
                                                v_t[:], 0.0)
                    nc.vector.tensor_scalar_max(p_pair[prow:prow + 64, :],
                                                v_t[:], 0.0)
                    if h % 2 == 1:
                        em_t = elup.tile([128, 512], BF16, tag="emt")
                        nc.scalar.activation(em_t[:], m_pair[:], AF.Exp)
                        nc.vector.tensor_add(h2cT[:, h // 2, :],
                                             em_t[:], p_pair[:])

                if dbg and g == dbg_g:
                    nc.sync.dma_start(out=dbg_h2c.ap(), in_=h2cT[:])

                # ================= layer 2 =================
                # h_prime2 (n-major) [i, 16] + ones col -> hp2x bf16
                hp2x = small.tile([128, NCH, 17], BF16, tag="hp2x")
                for ic in range(NCH):
                    hp2_ps = ps_sm.tile([128, 17], F32, tag="smps")
                    for fc in range(NCH):
                        nc.tensor.matmul(hp2_ps[:, 0:16],
                                         h2cT[:, fc, ic * 128:(ic + 1) * 128],
                                         w2f_sb[:, fc, :],
                                         start=(fc == 0), stop=False)
                    nc.tensor.matmul(hp2_ps[:, 0:16],
                                     onesrow[:, ic * 128:(ic + 1) * 128],
                                     negcs_sb[:],
                                     start=False, stop=True)
                    nc.vector.tensor_copy(hp2x[:, ic, 0:16], hp2_ps[:, 0:16])
                nc.vector.memset(hp2x[:, :, 16:17], 1.0)

                # h_prime2T [16, n] -> tanh -> t2 bf16
                hp2T_ps = ps_row.tile([16, 512], F32, tag="rowps")
                for fc in range(NCH):
                    nc.tensor.matmul(hp2T_ps[:], w2f_sb[:, fc, :],
                                     h2cT[:, fc, :],
                                     start=(fc == 0), stop=False)
                nc.tensor.matmul(hp2T_ps[:], negcs_sb[:], onesrow[:],
                                 start=False, stop=True)
                t2_sb = small.tile([16, 512], BF16, tag="t2")
                nc.scalar.activation(t2_sb[:], hp2T_ps[:], AF.Tanh)

                # s2 row and d2 cols
                s2_ps = ps_row.tile([16, 512], F32, tag="rowps")
                nc.tensor.matmul(s2_ps[0:1, :], a2s_sb[:], t2_sb[:],
                                 start=True, stop=True)
                s2_sb = small.tile([1, 512], F32, tag="s2")
                nc.vector.tensor_copy(s2_sb[:], s2_ps[0:1, :])
                d2_sb = small.tile([128, NCH], F32, tag="d2")
                for jc in range(NCH):
                    d2_ps = ps_sm.tile([128, 17], F32, tag="smps")
                    nc.tensor.matmul(d2_ps[:, 0:1],
                                     t2_sb[:, jc * 128:(jc + 1) * 128],
                                     a2d_sb[:], start=True, stop=True)
                    nc.vector.tensor_copy(d2_sb[:, jc:jc + 1], d2_ps[:, 0:1])

                # attention layer 2 (single head)
                s2_bc = sbcp.tile([128, 512], F32, tag="s_bc")
                nc.gpsimd.partition_broadcast(s2_bc[:], s2_sb[:])
                z2_all = attn.tile([128, NCH * 512], BF16, tag="z")
                for jc in range(NCH):
                    nc.scalar.activation(z2_all[:, jc * 512:(jc + 1) * 512],
                                         s2_bc[:], AF.Prelu,
                                         bias=d2_sb[:, jc:jc + 1],
                                         scale=1.0, alpha=0.2)
                e2_all = attn.tile([128, NCH * 512], BF16, tag="e")
                e2_inst = nc.scalar.activation(e2_all[:], z2_all[:], AF.Exp)
                if g == GPC - 1:
                    gate_act = e2_inst
                em2_all = attn.tile([128, NCH * 512], BF16, tag="em")
                nc.vector.tensor_mul(em2_all[:], e2_all[:], adjT_sb[:])

                # out2 [i, 17] per i-chunk; logits = out2[:, :16] / den2
                for ic in range(NCH):
                    o2_ps = ps_sm.tile([128, 17], F32, tag="smps")
                    for jc in range(NCH):
                        nc.tensor.matmul(
                            o2_ps[:],
                            em2_all[:, jc * 512 + ic * 128:
                                    jc * 512 + (ic + 1) * 128],
                            hp2x[:, jc, :],
                            start=(jc == 0), stop=(jc == NCH - 1))
                    r2 = small.tile([128, 1], F32, tag="r2")
                    nc.vector.reciprocal(r2[:], o2_ps[:, 16:17])
                    nc.vector.tensor_scalar_mul(logits_all[:, g, ic, :],
                                                o2_ps[:, 0:16], r2[:])
                    if dbg and g == dbg_g:
                        nc.sync.dma_start(out=dbg_lg.ap()[:, ic, :],
                                          in_=logits_all[:, g, ic, :])

            # ---------- deferred log_softmax ----------
            # Exps (same table set as attention, can fill bubbles anywhere);
            # all 16 accumulator sums land in one [128,16] tile so a single
            # Ln (pinned after the last graph) converts them; subs on DVE.
            S_all = finp.tile([128, GPC * NCH], F32)
            for g in range(GPC):
                for ic in range(NCH):
                    ex = small.tile([128, 16], F32, tag="ex")
                    idx = g * NCH + ic
                    nc.scalar.activation(ex[:], logits_all[:, g, ic, :],
                                         AF.Exp,
                                         accum_out=S_all[:, idx:idx + 1])
            L_all = finp.tile([128, GPC * NCH], F32)
            ln_i = nc.scalar.activation(L_all[:], S_all[:], AF.Ln)
            add_dep_helper(ln_i.ins, gate_act.ins, False,
                           "defer ln table switch")
            for g in range(GPC):
                for ic in range(NCH):
                    idx = g * NCH + ic
                    fin = small.tile([128, 16], F32, tag="fin")
                    nc.vector.tensor_scalar_sub(fin[:],
                                                logits_all[:, g, ic, :],
                                                L_all[:, idx:idx + 1])
                    nc.sync.dma_start(
                        out=out_d.ap()[g, ic * 128:(ic + 1) * 128, :],
                        in_=fin[:])
    return nc


def host_prep(adj, vertices, local_emb, emb0, emb1, w1, a_src1, a_dst1,
              w2, a_src2, a_dst2):
    """Build the 8 per-core input maps from full inputs."""
    adj = np.asarray(adj, dtype=np.float32)
    vertices = np.asarray(vertices)
    local_emb = np.asarray(local_emb, dtype=np.float32)
    emb0 = np.asarray(emb0, dtype=np.float32)
    emb1 = np.asarray(emb1, dtype=np.float32)
    w1 = np.asarray(w1, dtype=np.float32)
    a_src1 = np.asarray(a_src1, dtype=np.float32)
    a_dst1 = np.asarray(a_dst1, dtype=np.float32)
    w2 = np.asarray(w2, dtype=np.float32)
    a_src2 = np.asarray(a_src2, dtype=np.float32)
    a_dst2 = np.asarray(a_dst2, dtype=np.float32)

    vtx = vertices.astype(np.int64)
    # h: [b, n, 160] -> hT [b, 160, n]
    h = np.concatenate([emb0[vtx], emb1[vtx], local_emb], axis=2)
    hT = np.ascontiguousarray(h.transpose(0, 2, 1))

    # adjT packed: [b, 128, 4*512] bf16, block jc = adjT rows jc*128..
    adjT = adj.transpose(0, 2, 1)
    adjTp = np.ascontiguousarray(
        adjT.reshape(BS, NCH, 128, N).transpose(0, 2, 1, 3).reshape(
            BS, 128, NCH * N)).astype(BFNP)

    w1f = np.ascontiguousarray(w1.transpose(1, 0, 2).reshape(FIN1, 512))
    asrc = np.zeros((512, NH), np.float32)
    adst = np.zeros((512, NH), np.float32)
    for hh in range(NH):
        asrc[hh * 64:(hh + 1) * 64, hh] = a_src1[hh, :, 0]
        adst[hh * 64:(hh + 1) * 64, hh] = a_dst1[hh, :, 0]
    asrep = np.zeros((NCH, NH, 128, 128), np.float32)
    for oc in range(NCH):
        for hh in range(NH):
            asrep[oc, hh] = asrc[oc * 128:(oc + 1) * 128, hh:hh + 1]
    consts = {
        "w1f": w1f,
        "asrep": asrep.astype(BFNP),
        "asrc": asrc.astype(BFNP),
        "adst": adst.astype(BFNP),
        "w2f": w2[0].astype(BFNP),
        "negcs": (-w2[0].sum(axis=0, keepdims=True)).astype(BFNP),
        "a2s": a_src2[0].astype(BFNP),
        "a2d": a_dst2[0].astype(BFNP),
    }
    in_maps = []
    for core in range(NCORES):
        sl = slice(core * GPC, (core + 1) * GPC)
        m = dict(consts)
        m["hT4"] = np.ascontiguousarray(hT[sl])
        m["adjTp"] = np.ascontiguousarray(adjTp[sl])
        in_maps.append(m)
    return in_maps


_NC_CACHE = {}


def _get_nc():
    if "nc" not in _NC_CACHE:
        nc = build_nc()
        nc.compile()
        _NC_CACHE["nc"] = nc
    return _NC_CACHE["nc"]


def kernel(**inputs):
    from concourse.bass_utils import run_bass_kernel_spmd
    nc = _get_nc()
    in_maps = host_prep(**inputs)
    res = run_bass_kernel_spmd(nc, in_maps, core_ids=list(range(NCORES)))
    out = np.concatenate([r["out"] for r in res.results], axis=0)
    return out.astype(np.float32)


if __name__ == "__main__":
    nc = build_nc()
    print("built ok")



# revision 16
# speedup vs baseline: 1.2234x; 1.2234x over previous
"""TRN2 Bass kernel for nn_BatchDenseGAT (2-layer dense GAT, bs=32, n=512).

Sharding: data-parallel over the 32 graphs -> 4 graphs per NeuronCore x 8
cores, params replicated. Host does embedding gather/concat/transpose and
mask packing; all model math runs on device.

Device strategy per graph:
  exp(leaky_relu(s_i + d_j)) is factorized: with r_i = exp(-0.8*s_i),
  ed_j = exp(d_j), ed2_j = exp(0.2*d_j), the softmax-equivalent
  (row-normalization cancels any per-i factor) masked weight is
    em[j,i] = adjT[j,i] * max(r_i*ed2_j, ed_j)
  built with ONE dual-op tensor_scalar per [128,512] tile (DVE 4x mode)
  plus one bf16 mask multiply per head -- this removes the per-element
  Prelu+Exp work from the scalar engine entirely; only one exp per head
  ([128,512] r_bc from the PE-replicated s broadcast) remains on ACT.
  out1T per head is [65,512] with a ones-column folded into the matmul
  lhsT giving the softmax denominator in row 64. Denominators are
  reciprocal'd per head-pair, partition-broadcast on GpSimd, and the
  normalization multiply runs on GpSimd to keep DVE free. elu uses
  elu(v) = min(exp(v),1) + max(v,0) - 1 ("-1" folded into layer-2
  weights via negcs), with the min/add fused in one scalar_tensor_tensor.
  Layer 2 repeats the same factorized attention with one head and ends
  in a free-dim log_softmax (deferred exp/ln as in the baseline).
"""
import os
import sys
import numpy as np

sys.path.insert(0, '/opt/trn_rl_repo')

import ml_dtypes  # noqa: E402
import concourse.bacc as bacc  # noqa: E402
import concourse.bass as bass  # noqa: E402
import concourse.tile as tile  # noqa: E402
from concourse import mybir  # noqa: E402

F32 = mybir.dt.float32
BF16 = mybir.dt.bfloat16
AF = mybir.ActivationFunctionType
ALU = mybir.AluOpType
BFNP = ml_dtypes.bfloat16

BS, N, NH, FO = 32, 512, 8, 64
FIN1 = 160
NCORES = 8
GPC = BS // NCORES  # graphs per core
NCH = 4             # 512 / 128 partition chunks


def build_nc():
    B = 1 if os.environ.get("GAT_SERIAL", "0") == "1" else None
    dbg = os.environ.get("GAT_DEBUG_DUMP", "0") == "1"
    nc = bacc.Bacc("TRN2", target_bir_lowering=False, debug=False)

    hT4 = nc.dram_tensor("hT4", [GPC, FIN1, N], BF16, kind="ExternalInput")
    adjTp = nc.dram_tensor("adjTp", [GPC, 128, NCH * N], BF16,
                           kind="ExternalInput")
    w1f_d = nc.dram_tensor("w1f", [FIN1, 512], BF16, kind="ExternalInput")
    asrep_d = nc.dram_tensor("asrep", [NCH, NH, 128, 128], BF16,
                             kind="ExternalInput")
    adst_d = nc.dram_tensor("adst", [512, NH], BF16, kind="ExternalInput")
    w2f_d = nc.dram_tensor("w2f", [512, 16], BF16, kind="ExternalInput")
    negcs_d = nc.dram_tensor("negcs", [1, 16], BF16, kind="ExternalInput")
    a2srep_d = nc.dram_tensor("a2srep", [16, 128], BF16, kind="ExternalInput")
    a2d_d = nc.dram_tensor("a2d", [16, 1], BF16, kind="ExternalInput")
    out_d = nc.dram_tensor("out", [GPC, N, 16], F32, kind="ExternalOutput")
    if dbg:
        dbg_tT = nc.dram_tensor("dbg_tT", [128, NCH, 512], BF16, kind="ExternalOutput")
        dbg_d = nc.dram_tensor("dbg_d", [128, NCH, NH], F32, kind="ExternalOutput")
        dbg_ed = nc.dram_tensor("dbg_ed", [128, NCH, NH], F32, kind="ExternalOutput")
        dbg_r = nc.dram_tensor("dbg_r", [128, 512], BF16, kind="ExternalOutput")
        dbg_z = nc.dram_tensor("dbg_z", [128, NCH * 512], BF16, kind="ExternalOutput")
        dbg_em = nc.dram_tensor("dbg_em", [128, NCH * 512], BF16, kind="ExternalOutput")
        dbg_o65 = nc.dram_tensor("dbg_o65", [128, 512], F32, kind="ExternalOutput")
        dbg_rden = nc.dram_tensor("dbg_rden", [1, 512], F32, kind="ExternalOutput")
        dbg_v = nc.dram_tensor("dbg_v", [128, 512], BF16, kind="ExternalOutput")
        dbg_h2c = nc.dram_tensor("dbg_h2c", [128, NCH, 512], BF16, kind="ExternalOutput")
        dbg_t2 = nc.dram_tensor("dbg_t2", [16, 512], BF16, kind="ExternalOutput")
        dbg_lg = nc.dram_tensor("dbg_lg", [128, NCH, 16], F32, kind="ExternalOutput")

    with tile.TileContext(nc) as tc:
        with tc.tile_pool(name="consts", bufs=1) as consts, \
             tc.tile_pool(name="gbuf", bufs=(B or 3)) as gbuf, \
             tc.tile_pool(name="attn", bufs=(B or 3)) as attn, \
             tc.tile_pool(name="small", bufs=(B or 2)) as small, \
             tc.tile_pool(name="elu", bufs=(B or 3)) as elup, \
             tc.tile_pool(name="fin", bufs=1) as finp, \
             tc.tile_pool(name="ps_big", bufs=(B or 3), space="PSUM") as ps_big, \
             tc.tile_pool(name="ps_row", bufs=1, space="PSUM") as ps_row, \
             tc.tile_pool(name="ps_sm", bufs=(B or 2), space="PSUM") as ps_sm, \
             tc.tile_pool(name="ps_sbc", bufs=(B or 2), space="PSUM") as ps_sbc:

            # ---------- constants ----------
            w1f_a = consts.tile([128, 512], BF16)
            w1f_b = consts.tile([32, 512], BF16)
            nc.sync.dma_start(out=w1f_a, in_=w1f_d.ap()[0:128, :])
            nc.sync.dma_start(out=w1f_b, in_=w1f_d.ap()[128:160, :])
            asrep_sb = consts.tile([128, NCH, NH, 128], BF16)
            nc.gpsimd.dma_start(out=asrep_sb,
                                in_=asrep_d.ap().rearrange("c a p q -> p c a q"))
            adst_sb = consts.tile([128, NCH, NH], BF16)
            nc.sync.dma_start(out=adst_sb,
                              in_=adst_d.ap().rearrange("(c p) a -> p c a", c=NCH))
            w2f_sb = consts.tile([128, NCH, 16], BF16)
            nc.sync.dma_start(out=w2f_sb,
                              in_=w2f_d.ap().rearrange("(c p) a -> p c a", c=NCH))
            negcs_sb = consts.tile([1, 16], BF16)
            nc.sync.dma_start(out=negcs_sb, in_=negcs_d.ap())
            a2srep_sb = consts.tile([16, 128], BF16)
            nc.sync.dma_start(out=a2srep_sb, in_=a2srep_d.ap())
            a2d_sb = consts.tile([16, 1], BF16)
            nc.sync.dma_start(out=a2d_sb, in_=a2d_d.ap())
            onesrow = consts.tile([1, 512], BF16)
            nc.vector.memset(onesrow, 1.0)

            logits_all = finp.tile([128, GPC, NCH, 16], F32)

            for g in range(GPC):
                # ---------- graph loads ----------
                hT_a = gbuf.tile([128, N], BF16, tag="hT_a")
                hT_b = gbuf.tile([32, N], BF16, tag="hT_b")
                nc.sync.dma_start(out=hT_a, in_=hT4.ap()[g, 0:128, :])
                nc.sync.dma_start(out=hT_b, in_=hT4.ap()[g, 128:160, :])
                adjT_sb = gbuf.tile([128, NCH * N], BF16, tag="adjT")
                nc.sync.dma_start(out=adjT_sb, in_=adjTp.ap()[g])

                # -- h_prime (n-major) -> hpx bf16 [i, (ic), (h, 64+ones)] --
                hpx = gbuf.tile([128, NCH, NH, 65], BF16, tag="hpx")
                nc.vector.memset(hpx[:, :, :, 64:65], 1.0)
                for ic in range(NCH):
                    hp_ps = ps_sbc.tile([128, 512], F32, tag="sbcps")
                    nc.tensor.matmul(hp_ps[:], hT_a[:, ic * 128:(ic + 1) * 128],
                                     w1f_a[:], start=True, stop=False)
                    nc.tensor.matmul(hp_ps[:], hT_b[:, ic * 128:(ic + 1) * 128],
                                     w1f_b[:], start=False, stop=True)
                    nc.scalar.activation(
                        hpx[:, ic, :, 0:64],
                        hp_ps[:].rearrange("p (h o) -> p h o", h=NH), AF.Copy)

                # ---------- h_primeT (o-major) -> tanh -> tT bf16 ----------
                tT = gbuf.tile([128, NCH, 512], BF16, tag="tT")
                for oc in range(NCH):
                    hpT_ps = ps_sbc.tile([128, 512], F32, tag="sbcps")
                    nc.tensor.matmul(hpT_ps[:],
                                     w1f_a[:, oc * 128:(oc + 1) * 128],
                                     hT_a[:], start=True, stop=False)
                    nc.tensor.matmul(hpT_ps[:],
                                     w1f_b[:, oc * 128:(oc + 1) * 128],
                                     hT_b[:], start=False, stop=True)
                    nc.scalar.activation(tT[:, oc, :], hpT_ps[:], AF.Tanh)

                # ---------- d cols + exp'd scalars ----------
                d_sb = small.tile([128, NCH, NH], F32, tag="d_sb")
                for jc in range(NCH):
                    d_ps = ps_sbc.tile([128, 512], F32, tag="sbcps")
                    for oc in range(NCH):
                        nc.tensor.matmul(d_ps[:, 0:NH],
                                         tT[:, oc, jc * 128:(jc + 1) * 128],
                                         adst_sb[:, oc, :],
                                         start=(oc == 0), stop=(oc == NCH - 1))
                    nc.vector.tensor_copy(d_sb[:, jc, :], d_ps[:, 0:NH])
                ed_sb = small.tile([128, NCH, NH], F32, tag="ed")
                nc.scalar.activation(ed_sb[:], d_sb[:], AF.Exp)
                ed2_sb = small.tile([128, NCH, NH], F32, tag="ed2")
                nc.scalar.activation(ed2_sb[:], d_sb[:], AF.Exp, scale=0.2)
                if dbg and g == 0:
                    nc.sync.dma_start(out=dbg_tT.ap(), in_=tT[:])
                    nc.sync.dma_start(out=dbg_d.ap(), in_=d_sb[:])
                    nc.sync.dma_start(out=dbg_ed.ap(), in_=ed_sb[:])

                # ---------- per-head attention ----------
                # out1T per head is [65, 512]: rows 0-63 = hp_h.T @ em,
                # row 64 = denominator (ones column folded into lhsT).
                h2cT = gbuf.tile([128, NCH, 512], BF16, tag="h2cT")
                v_pair = None
                for h in range(NH):
                    # s broadcast from PE: each lhsT column is the same
                    # a_src chunk, so the matmul emits s replicated across
                    # all 128 partitions.
                    s_bc = ps_sbc.tile([128, 512], F32, tag="sbcps")
                    for oc in range(NCH):
                        nc.tensor.matmul(s_bc[:], asrep_sb[:, oc, h, :],
                                         tT[:, oc, :],
                                         start=(oc == 0), stop=(oc == NCH - 1))
                    r_bc = attn.tile([128, 512], BF16, tag="rbc")
                    nc.scalar.activation(r_bc[:], s_bc[:], AF.Exp, scale=-0.8)
                    z_all = attn.tile([128, NCH * 512], BF16, tag="z")
                    for jc in range(NCH):
                        nc.vector.tensor_scalar(
                            z_all[:, jc * 512:(jc + 1) * 512], r_bc[:],
                            ed2_sb[:, jc, h:h + 1], ed_sb[:, jc, h:h + 1],
                            op0=ALU.mult, op1=ALU.max)
                    em_all = attn.tile([128, NCH * 512], BF16, tag="em")
                    nc.vector.tensor_mul(em_all[:], z_all[:], adjT_sb[:])
                    if dbg and g == 0 and h == 0:
                        nc.sync.dma_start(out=dbg_r.ap(), in_=r_bc[:])
                        nc.sync.dma_start(out=dbg_z.ap(), in_=z_all[:])
                        nc.sync.dma_start(out=dbg_em.ap(), in_=em_all[:])

                    o65_ps = ps_big.tile([128, 512], F32, tag="bigps")
                    for jc in range(NCH):
                        nc.tensor.matmul(
                            o65_ps[0:65, :], hpx[:, jc, h, :],
                            em_all[:, jc * 512:(jc + 1) * 512],
                            start=(jc == 0), stop=(jc == NCH - 1))
                    # -------- normalize: v = out1 / den --------
                    rden_t = elup.tile([1, 512], F32, tag="rden")
                    nc.vector.reciprocal(rden_t[:], o65_ps[64:65, :])
                    if dbg and g == 0 and h == 0:
                        o65_sb = elup.tile([128, 512], F32, tag="dbg65")
                        nc.vector.tensor_copy(o65_sb[0:65, :], o65_ps[0:65, :])
                        nc.sync.dma_start(out=dbg_o65.ap()[0:65, :],
                                          in_=o65_sb[0:65, :])
                        nc.sync.dma_start(out=dbg_rden.ap(), in_=rden_t[:])
                    prow = (h % 2) * 64
                    if h % 2 == 0:
                        v_pair = elup.tile([128, 512], BF16, tag="v")
                    rb_t = elup.tile([64, 512], F32, tag="rb")
                    nc.gpsimd.partition_broadcast(rb_t[:], rden_t[:])
                    nc.vector.tensor_mul(v_pair[prow:prow + 64, :],
                                         o65_ps[0:64, :], rb_t[:])
                    if h % 2 == 0:
                        continue
                    # elu(v) = min(exp(v),1) + max(v,0) - 1 (the -1 lives in
                    # negcs folded into layer-2 weights)
                    ev = elup.tile([128, 512], BF16, tag="ev")
                    nc.scalar.activation(ev[:], v_pair[:], AF.Exp)
                    p_pair = elup.tile([128, 512], BF16, tag="p")
                    nc.vector.tensor_scalar_max(p_pair[:], v_pair[:], 0.0)
                    nc.vector.scalar_tensor_tensor(h2cT[:, h // 2, :], ev[:],
                                                   1.0, p_pair[:],
                                                   op0=ALU.min, op1=ALU.add)
                    if dbg and g == 0 and h == 1:
                        nc.sync.dma_start(out=dbg_v.ap(), in_=v_pair[:])

                if dbg and g == 0:
                    nc.sync.dma_start(out=dbg_h2c.ap(), in_=h2cT[:])

                # ================= layer 2 =================
                # h_prime2 (n-major) [i, 16] + ones col -> hp2x bf16
                hp2x = small.tile([128, NCH, 17], BF16, tag="hp2x")
                for ic in range(NCH):
                    hp2_ps = ps_sm.tile([128, 17], F32, tag="smps")
                    for fc in range(NCH):
                        nc.tensor.matmul(hp2_ps[:, 0:16],
                                         h2cT[:, fc, ic * 128:(ic + 1) * 128],
                                         w2f_sb[:, fc, :],
                                         start=(fc == 0), stop=False)
                    nc.tensor.matmul(hp2_ps[:, 0:16],
                                     onesrow[:, ic * 128:(ic + 1) * 128],
                                     negcs_sb[:],
                                     start=False, stop=True)
                    nc.vector.tensor_copy(hp2x[:, ic, 0:16], hp2_ps[:, 0:16])
                nc.vector.memset(hp2x[:, :, 16:17], 1.0)

                # h_prime2T [16, n] -> tanh -> t2 bf16
                hp2T_ps = ps_row.tile([16, 512], F32, tag="rowps")
                for fc in range(NCH):
                    nc.tensor.matmul(hp2T_ps[:], w2f_sb[:, fc, :],
                                     h2cT[:, fc, :],
                                     start=(fc == 0), stop=False)
                nc.tensor.matmul(hp2T_ps[:], negcs_sb[:], onesrow[:],
                                 start=False, stop=True)
                t2_sb = small.tile([16, 512], BF16, tag="t2")
                nc.scalar.activation(t2_sb[:], hp2T_ps[:], AF.Tanh)
                if dbg and g == 0:
                    nc.sync.dma_start(out=dbg_t2.ap(), in_=t2_sb[:])

                # s2 broadcast via replicated-column matmul; d2 cols
                s2_ps = ps_sbc.tile([128, 512], F32, tag="sbcps")
                nc.tensor.matmul(s2_ps[:], a2srep_sb[:], t2_sb[:],
                                 start=True, stop=True)
                r2_bc = attn.tile([128, 512], BF16, tag="rbc")
                nc.scalar.activation(r2_bc[:], s2_ps[:], AF.Exp, scale=-0.8)
                d2_sb = small.tile([128, NCH], F32, tag="d2")
                for jc in range(NCH):
                    d2_ps = ps_sm.tile([128, 17], F32, tag="smps")
                    nc.tensor.matmul(d2_ps[:, 0:1],
                                     t2_sb[:, jc * 128:(jc + 1) * 128],
                                     a2d_sb[:], start=True, stop=True)
                    nc.vector.tensor_copy(d2_sb[:, jc:jc + 1], d2_ps[:, 0:1])
                e2d_sb = small.tile([128, NCH], F32, tag="e2d")
                nc.scalar.activation(e2d_sb[:], d2_sb[:], AF.Exp)
                e2d2_sb = small.tile([128, NCH], F32, tag="e2d2")
                nc.scalar.activation(e2d2_sb[:], d2_sb[:], AF.Exp, scale=0.2)

                # attention layer 2 (single head), factorized as layer 1
                z2_all = attn.tile([128, NCH * 512], BF16, tag="z")
                for jc in range(NCH):
                    nc.vector.tensor_scalar(
                        z2_all[:, jc * 512:(jc + 1) * 512], r2_bc[:],
                        e2d2_sb[:, jc:jc + 1], e2d_sb[:, jc:jc + 1],
                        op0=ALU.mult, op1=ALU.max)
                em2_all = attn.tile([128, NCH * 512], BF16, tag="em")
                nc.vector.tensor_mul(em2_all[:], z2_all[:], adjT_sb[:])

                # out2 [i, 17] per i-chunk; logits = out2[:, :16] / den2
                for ic in range(NCH):
                    o2_ps = ps_sm.tile([128, 17], F32, tag="smps")
                    for jc in range(NCH):
                        nc.tensor.matmul(
                            o2_ps[:],
                            em2_all[:, jc * 512 + ic * 128:
                                    jc * 512 + (ic + 1) * 128],
                            hp2x[:, jc, :],
                            start=(jc == 0), stop=(jc == NCH - 1))
                    r2c = small.tile([128, 1], F32, tag="r2c")
                    nc.vector.reciprocal(r2c[:], o2_ps[:, 16:17])
                    nc.vector.tensor_scalar_mul(logits_all[:, g, ic, :],
                                                o2_ps[:, 0:16], r2c[:])
                    if dbg and g == 0:
                        nc.sync.dma_start(out=dbg_lg.ap()[:, ic, :],
                                          in_=logits_all[:, g, ic, :])

            # ---------- deferred log_softmax ----------
            S_all = finp.tile([128, GPC * NCH], F32)
            for g in range(GPC):
                for ic in range(NCH):
                    ex = small.tile([128, 16], F32, tag="ex")
                    idx = g * NCH + ic
                    nc.scalar.activation(ex[:], logits_all[:, g, ic, :],
                                         AF.Exp,
                                         accum_out=S_all[:, idx:idx + 1])
            L_all = finp.tile([128, GPC * NCH], F32)
            nc.scalar.activation(L_all[:], S_all[:], AF.Ln)
            for g in range(GPC):
                for ic in range(NCH):
                    idx = g * NCH + ic
                    fin = small.tile([128, 16], F32, tag="fin")
                    nc.vector.tensor_scalar_sub(fin[:],
                                                logits_all[:, g, ic, :],
                                                L_all[:, idx:idx + 1])
                    nc.sync.dma_start(
                        out=out_d.ap()[g, ic * 128:(ic + 1) * 128, :],
                        in_=fin[:])
    return nc


def host_prep(adj, vertices, local_emb, emb0, emb1, w1, a_src1, a_dst1,
              w2, a_src2, a_dst2):
    """Build the 8 per-core input maps from full inputs."""
    adj = np.asarray(adj, dtype=np.float32)
    vertices = np.asarray(vertices)
    local_emb = np.asarray(local_emb, dtype=np.float32)
    emb0 = np.asarray(emb0, dtype=np.float32)
    emb1 = np.asarray(emb1, dtype=np.float32)
    w1 = np.asarray(w1, dtype=np.float32)
    a_src1 = np.asarray(a_src1, dtype=np.float32)
    a_dst1 = np.asarray(a_dst1, dtype=np.float32)
    w2 = np.asarray(w2, dtype=np.float32)
    a_src2 = np.asarray(a_src2, dtype=np.float32)
    a_dst2 = np.asarray(a_dst2, dtype=np.float32)

    vtx = vertices.astype(np.int64)
    # h: [b, n, 160] -> hT [b, 160, n]
    h = np.concatenate([emb0[vtx], emb1[vtx], local_emb], axis=2)
    hT = np.ascontiguousarray(h.transpose(0, 2, 1)).astype(BFNP)

    # adjT packed: [b, 128, 4*512] bf16, block jc = adjT rows jc*128..
    adjT = adj.transpose(0, 2, 1)
    adjTp = np.ascontiguousarray(
        adjT.reshape(BS, NCH, 128, N).transpose(0, 2, 1, 3).reshape(
            BS, 128, NCH * N)).astype(BFNP)

    w1f = np.ascontiguousarray(w1.transpose(1, 0, 2).reshape(FIN1, 512))
    asrc = np.zeros((512, NH), np.float32)
    adst = np.zeros((512, NH), np.float32)
    for hh in range(NH):
        asrc[hh * 64:(hh + 1) * 64, hh] = a_src1[hh, :, 0]
        adst[hh * 64:(hh + 1) * 64, hh] = a_dst1[hh, :, 0]
    asrep = np.zeros((NCH, NH, 128, 128), np.float32)
    for oc in range(NCH):
        for hh in range(NH):
            asrep[oc, hh] = asrc[oc * 128:(oc + 1) * 128, hh:hh + 1]
    a2srep = np.repeat(a_src2[0], 128, axis=1)  # [16, 128]
    consts = {
        "w1f": w1f.astype(BFNP),
        "asrep": asrep.astype(BFNP),
        "adst": adst.astype(BFNP),
        "w2f": w2[0].astype(BFNP),
        "negcs": (-w2[0].sum(axis=0, keepdims=True)).astype(BFNP),
        "a2srep": a2srep.astype(BFNP),
        "a2d": a_dst2[0].astype(BFNP),
    }
    in_maps = []
    for core in range(NCORES):
        sl = slice(core * GPC, (core + 1) * GPC)
        m = dict(consts)
        m["hT4"] = np.ascontiguousarray(hT[sl])
        m["adjTp"] = np.ascontiguousarray(adjTp[sl])
        in_maps.append(m)
    return in_maps


_NC_CACHE = {}


def _get_nc():
    if "nc" not in _NC_CACHE:
        nc = build_nc()
        nc.compile()
        _NC_CACHE["nc"] = nc
    return _NC_CACHE["nc"]


def kernel(**inputs):
    from concourse.bass_utils import run_bass_kernel_spmd
    nc = _get_nc()
    in_maps = host_prep(**inputs)
    res = run_bass_kernel_spmd(nc, in_maps, core_ids=list(range(NCORES)))
    out = np.concatenate([r["out"] for r in res.results], axis=0)
    return out.astype(np.float32)


if __name__ == "__main__":
    nc = build_nc()
    print("built ok")
